# revision 18
# baseline (speedup 1.0000x reference)
"""Trainium2 Bass kernel for dynamic low-pass filter decomposition.

Module: global-avg-pool -> 1x1 conv -> BN -> softmax over 3x3 taps gives a
per-(sample, group) 3x3 kernel; applied as a reflect-padded depthwise conv
over x; returns (low, x - low).

Sharding: 16 half-sample shards over 8 NeuronCores, pipelined.  Cores 2j and
2j+1 co-own samples 2j (shard A: core 2j has rows 0..95, core 2j+1 rows
96..191) and 2j+1 (shard B, halves swapped).  Because the softmax kernel
depends on the sample's global mean, each pair AllReduces its [128,1]
partial sums (512B) before computing; shard B's input load and weight chain
overlap shard A's compute, hiding most of the otherwise-serial input phase.

Per-shard layout: partition p = c*2 + b (b = 48-row block of the half,
c = channel).  The host pads each shard to 98 rows so every partition loads
one contiguous 50-row run (48 rows + both halo rows, reflection resolved by
host-side row choice).

Engine split: the TensorEngine accumulates 8 of the 9 taps as diagonal fp32r
matmuls per 512-col PSUM chunk; ScalarE copies the partial out of PSUM;
VectorE adds the center tap in-place in SBUF, fixes the reflect columns at
w=0/191 (6 merged ops) and computes high = x - low.  Consts load on the
GpSimd DMA queue; warm-up matmuls keep the PE p-state ramped through the
input phase.
"""
import sys
import os

sys.path.insert(0, "/opt/trn_rl_repo")

import numpy as np
from contextlib import ExitStack

import concourse.bass as bass
import concourse.tile as tile
from concourse import bacc, mybir
from concourse.bass_utils import run_bass_kernel_spmd

dt = mybir.dt
f32 = dt.float32

KS = 3
GROUP = 8
IC = 64
BN_EPS = 1e-5
N = 8
H = W = 192
RB = 48                 # rows per partition block (half image = 2 blocks)
NBS = 50 * W            # shard buffer elems per partition (48 rows + halos)
SROWS = 98              # host-padded rows per shard input tensor
PAD = 1                 # front pad elems (also 1 at the back)
CH = 512                # matmul chunk (one PSUM bank)
# per-shard load chunks (offset, size) in buffer elems, alternating queues
# with balanced bytes; sizes shrink so the last partial-sum lands early
LOAD_CHUNKS = [(0, 3840), (3840, 3840), (7680, 960), (8640, 960)]
SYNC_LOADS = (0, 2)
WARM_PER_CHUNK = {0: 3, 1: 3, 2: 1, 3: 1}
WARM_BRIDGE = 4
WARM_CHAIN = 2
ST_ROWS_A = [16, 16, 8, 8]
ST_ROWS_B = [16, 16, 8, 4, 4]
PE_ALL9_B = (3, 4)             # B drain STs: PE does all 9 taps
PE_TAPS = (0, 1, 2, 3, 5, 6, 7, 8)
V_TAP = 4                      # center tap, added in-place in SBUF
REPLICA_GROUPS = [[0, 1], [2, 3], [4, 5], [6, 7]]


def _build_program():
    """Trace the SPMD Bass program (same for every core)."""
    nc = bacc.Bacc("TRN2", target_bir_lowering=False, debug=False,
                   num_devices=N)

    xa_d = nc.dram_tensor("xa", [64, SROWS, W], dt.float32r,
                          kind="ExternalInput")
    xb_d = nc.dram_tensor("xb", [64, SROWS, W], dt.float32r,
                          kind="ExternalInput")
    at_d = nc.dram_tensor("at128", [128, 72], f32, kind="ExternalInput")
    b_d = nc.dram_tensor("b72", [72, 1], f32, kind="ExternalInput")
    r9_d = nc.dram_tensor("r9", [72, 9], f32, kind="ExternalInput")
    g_d = nc.dram_tensor("g728", [72, 8], f32, kind="ExternalInput")
    h_d = nc.dram_tensor("h8128", [8, 128], f32, kind="ExternalInput")
    eye_d = nc.dram_tensor("eye", [128, 128], f32, kind="ExternalInput")
    eyer_d = nc.dram_tensor("eyer", [128, 128], dt.float32r,
                            kind="ExternalInput")
    outs_d = {}
    for s in "ab":
        outs_d[f"low_{s}"] = nc.dram_tensor(f"low_{s}", [64, 2 * RB, W], f32,
                                            kind="ExternalOutput")
        outs_d[f"high_{s}"] = nc.dram_tensor(f"high_{s}", [64, 2 * RB, W],
                                             f32, kind="ExternalOutput")
    cc_in = {s: nc.dram_tensor(f"ccin_{s}", [128, 1], f32, kind="Internal")
             for s in "ab"}
    cc_out = {s: nc.dram_tensor(f"ccout_{s}", [128, 1], f32, kind="Internal")
              for s in "ab"}

    def shard_load_ap(tensor, base, inner):
        """Per-partition contiguous 50-row run: partition p = c*2 + b reads
        x.flat[c*98*192 + b*48*192 + base : ... + inner]."""
        return bass.AP(tensor, base, [[SROWS * W, 64], [RB * W, 2],
                                      [1, inner]])

    def out_flat(tensor, base, inner):
        """Flat (128, inner) AP over a [64, 96, 192] output: partition
        p = c*2 + b covers rows 48b..48b+47 of channel c."""
        return bass.AP(tensor, base, [[RB * W, 128], [1, inner]])

    with tile.TileContext(nc) as tc, ExitStack() as ctx:
        cpool = ctx.enter_context(tc.tile_pool(name="consts", bufs=1))
        xpool = ctx.enter_context(tc.tile_pool(name="x", bufs=1))
        wpool = ctx.enter_context(tc.tile_pool(name="w", bufs=1))
        spool = ctx.enter_context(tc.tile_pool(name="stage", bufs=3))
        chainp = ctx.enter_context(tc.tile_pool(name="chainp", bufs=1,
                                                space=bass.MemorySpace.PSUM))

        # ---- shard loads FIRST (queue FIFO position = landing time);
        # consts go on the GpSimd queue so they never delay x ----
        xt = {"a": xpool.tile([128, PAD + NBS + 1], dt.float32r, name="xta"),
              "b": xpool.tile([128, PAD + NBS + 1], dt.float32r, name="xtb")}
        for t in xt.values():
            nc.vector.memset(t[:, 0:PAD].bitcast(f32), 0.0)
            nc.vector.memset(t[:, PAD + NBS:PAD + NBS + 1].bitcast(f32), 0.0)
        for s, d in (("a", xa_d), ("b", xb_d)):
            for i, (off, sz) in enumerate(LOAD_CHUNKS):
                eng = nc.sync if i in SYNC_LOADS else nc.scalar
                eng.dma_start(xt[s][:, PAD + off:PAD + off + sz],
                              shard_load_ap(d.ap().tensor, off, sz))

        at_s = cpool.tile([128, 72], f32)
        b_s = cpool.tile([72, 1], f32)
        r9_s = cpool.tile([72, 9], f32)
        g_s = cpool.tile([72, 8], f32)
        h_s = cpool.tile([8, 128], f32)
        eye_s = cpool.tile([128, 128], f32)
        eyer_s = cpool.tile([128, 128], dt.float32r)
        for t, d in ((eyer_s, eyer_d), (at_s, at_d), (b_s, b_d),
                     (r9_s, r9_d), (g_s, g_d), (h_s, h_d), (eye_s, eye_d)):
            nc.gpsimd.dma_start(t[:], d.ap())

        pt = {s: wpool.tile([128, 4], f32, name=f"pt{s}") for s in "ab"}
        rscratch = wpool.tile([128, 3840], f32)
        edummy = wpool.tile([72, 1], f32)
        ssum = {s: wpool.tile([128, 1], f32, name=f"ssum{s}") for s in "ab"}
        w128 = {s: wpool.tile([128, 9], f32, name=f"w128{s}") for s in "ab"}
        wcm = {s: wpool.tile([128, 3], f32, name=f"wc{s}") for s in "ab"}
        diag = {s: [wpool.tile([128, 128], dt.float32r, name=f"diag{s}{k}")
                    for k in range(9)] for s in "ab"}
        wrm = chainp.tile([128, CH], f32, tag="wrm")
        chn = chainp.tile([128, 16], f32, tag="chain")

        def partials(s, warm, table_warm=False):
            """Per-chunk partial sums (V reduce / S accumulate) + optional
            PE warm-up matmuls chained to each chunk's landing."""
            vcol = scol = 0
            for i, (off, sz) in enumerate(LOAD_CHUNKS):
                # the mean covers only the 48 interior rows (buffer rows
                # 1..48), not the halo rows at 0 and 49
                rlo, rhi = max(off, W), min(off + sz, NBS - W)
                src = xt[s][:, PAD + rlo:PAD + rhi].bitcast(f32)
                if i in SYNC_LOADS:
                    nc.vector.tensor_reduce(pt[s][:, vcol:vcol + 1], src,
                                            axis=mybir.AxisListType.X,
                                            op=mybir.AluOpType.add)
                    vcol += 1
                else:
                    nc.scalar.activation(rscratch[:, 0:rhi - rlo], src,
                                         mybir.ActivationFunctionType.Copy,
                                         accum_out=pt[s][:, 2 + scol:3 + scol])
                    scol += 1
                    if table_warm and scol == 1:
                        # dummy Exp: loads the activation table holding Exp
                        # off the chain's critical path
                        nc.scalar.activation(edummy[:], b_s[:],
                                             mybir.ActivationFunctionType.Exp)
                if warm:
                    a = PAD + off
                    for j in range(WARM_PER_CHUNK[i]):
                        nc.tensor.matmul(wrm[:], eyer_s[:],
                                         xt[s][:, a:a + CH])
            if warm:
                a = PAD + LOAD_CHUNKS[-1][0]
                for j in range(WARM_BRIDGE):
                    nc.tensor.matmul(wrm[:], eyer_s[:], xt[s][:, a:a + CH])

        def cc_exchange(s):
            """sum(pt) -> DRAM -> pair AllReduce -> SBUF ssum."""
            nc.vector.tensor_reduce(ssum[s][:], pt[s][:],
                                    axis=mybir.AxisListType.X,
                                    op=mybir.AluOpType.add)
            nc.gpsimd.dma_start(cc_in[s].ap(), ssum[s][:])
            nc.gpsimd.collective_compute(
                "AllReduce", mybir.AluOpType.add, REPLICA_GROUPS,
                [cc_in[s].ap()], [cc_out[s].ap()])
            nc.gpsimd.dma_start(ssum[s][:], cc_out[s].ap())

        def chain(s, warm):
            """softmax weight chain from the pair-summed ssum."""
            def warm_chain():
                if warm:
                    for j in range(WARM_CHAIN):
                        nc.tensor.matmul(wrm[:], eyer_s[:],
                                         xt[s][:, PAD:PAD + CH])
            lf_p = chn[0:72, 0:1]
            nc.tensor.matmul(lf_p, at_s[:], ssum[s][:])
            warm_chain()
            e72 = wpool.tile([72, 1], f32, name=f"e72{s}")
            nc.scalar.activation(e72[:], lf_p,
                                 mybir.ActivationFunctionType.Exp,
                                 bias=b_s[:, 0:1], scale=1.0)
            rhsw = wpool.tile([72, 9], f32, name=f"rhsw{s}")
            nc.vector.tensor_scalar_mul(rhsw[:], r9_s[:], e72[:, 0:1])
            w89_p = chn[0:8, 0:9]
            nc.tensor.matmul(w89_p, g_s[:], rhsw[:])
            warm_chain()
            s8 = wpool.tile([8, 1], f32, name=f"s8{s}")
            nc.vector.tensor_reduce(s8[:], w89_p, axis=mybir.AxisListType.X,
                                    op=mybir.AluOpType.add)
            r8 = wpool.tile([8, 1], f32, name=f"r8{s}")
            nc.vector.reciprocal(r8[:], s8[:])
            w89s = wpool.tile([8, 9], f32, name=f"w89s{s}")
            nc.vector.tensor_scalar_mul(w89s[:], w89_p, r8[:, 0:1])
            wbig_p = chn[:, 0:9]
            nc.tensor.matmul(wbig_p, h_s[:], w89s[:])
            warm_chain()
            nc.scalar.copy(w128[s][:], wbig_p)
            for k in PE_TAPS:
                nc.vector.tensor_scalar_mul(diag[s][k][:], eye_s[:],
                                            w128[s][:, k:k + 1])
            nc.vector.tensor_scalar_mul(diag[s][V_TAP][:], eye_s[:],
                                        w128[s][:, V_TAP:V_TAP + 1])
            nc.vector.tensor_tensor(wcm[s][:], w128[s][:, 0:9:3],
                                    w128[s][:, 2:9:3],
                                    op=mybir.AluOpType.add)

        def super_tile(s, st_idx, r0, rows, all9, psum):
            stw = rows * W
            base = PAD + W + r0 * W
            chunks = []
            o = 0
            while o < stw:
                chunks.append((o, min(CH, stw - o)))
                o += CH

            def tap_view(k, lo, sz):
                di, dj = k // 3, k % 3
                off = base + lo + (di - 1) * W + (dj - 1)
                return xt[s][:, off:off + sz]

            acc = [psum.tile([128, csz], f32, tag="acc",
                             name=f"acc{s}{st_idx}_{i}")
                   for i, (co, csz) in enumerate(chunks)]
            pe_taps = tuple(range(9)) if all9 else PE_TAPS
            taps = pe_taps if st_idx % 2 == 0 else pe_taps[::-1]
            for k in taps:
                for i, (co, csz) in enumerate(chunks):
                    nc.tensor.matmul(acc[i][:], diag[s][k][:],
                                     tap_view(k, co, csz),
                                     start=(k == taps[0]),
                                     stop=(k == taps[-1]))
            low_st = spool.tile([128, stw], f32, tag="low",
                                padded_shape=[128, 3072],
                                name=f"low{s}{st_idx}")
            for i, (co, csz) in enumerate(chunks):
                dst = low_st[:, co:co + csz]
                nc.scalar.copy(dst, acc[i][:])
                if not all9:
                    nc.vector.scalar_tensor_tensor(
                        dst, tap_view(V_TAP, co, csz).bitcast(f32),
                        w128[s][:, V_TAP:V_TAP + 1], dst,
                        op0=mybir.AluOpType.mult,
                        op1=mybir.AluOpType.add)
            out_ap = low_st[:, 0:stw].rearrange(
                "p (r w) -> p r w", w=W)[:, :, 0:W:W - 1]
            for di in range(3):
                vb = PAD + (r0 + di) * W
                va = xt[s][:, vb:vb + stw].bitcast(f32).rearrange(
                    "p (r w) -> p r w", w=W)[:, :, 0:W:W - 1]
                vn = xt[s][:, vb + 1:vb + 1 + stw].bitcast(f32).rearrange(
                    "p (r w) -> p r w", w=W)[:, :, 0:190:189]
                if di == 0:
                    nc.vector.tensor_scalar_mul(out_ap, va, w128[s][:, 1:2])
                else:
                    nc.vector.scalar_tensor_tensor(
                        out_ap, va, w128[s][:, 3 * di + 1:3 * di + 2],
                        out_ap, op0=mybir.AluOpType.mult,
                        op1=mybir.AluOpType.add)
                nc.vector.scalar_tensor_tensor(
                    out_ap, vn, wcm[s][:, di:di + 1], out_ap,
                    op0=mybir.AluOpType.mult,
                    op1=mybir.AluOpType.add)
            high_st = spool.tile([128, stw], f32, tag="high",
                                 padded_shape=[128, 3072],
                                 name=f"high{s}{st_idx}")
            nc.vector.tensor_tensor(high_st[:],
                                    xt[s][:, base:base + stw].bitcast(f32),
                                    low_st[:],
                                    op=mybir.AluOpType.subtract)
            nc.scalar.dma_start(
                out_flat(outs_d[f"low_{s}"].ap().tensor, r0 * W, stw),
                low_st[:])
            nc.sync.dma_start(
                out_flat(outs_d[f"high_{s}"].ap().tensor, r0 * W, stw),
                high_st[:])

        # ---- schedule ----
        partials("a", warm=True, table_warm=True)
        cc_exchange("a")
        chain("a", warm=True)
        # B's partial sums + AllReduce fill the engine-idle window while A's
        # first super-tiles run on the PE
        partials("b", warm=False)
        cc_exchange("b")

        with tc.tile_pool(name="psum", bufs=6,
                          space=bass.MemorySpace.PSUM) as psum:
            r0 = 0
            for st_idx, rows in enumerate(ST_ROWS_A):
                super_tile("a", st_idx, r0, rows, False, psum)
                r0 += rows
                if st_idx == 1:
                    # B's chain slots in here: its PE matmuls run in a ~1us
                    # bubble after A's second super-tile, its V ops fill V's
                    # idle window, well before the PE reaches B's taps
                    chain("b", warm=False)
            r0 = 0
            for st_idx, rows in enumerate(ST_ROWS_B):
                super_tile("b", st_idx, r0, rows, st_idx in PE_ALL9_B, psum)
                r0 += rows

    nc.compile()
    return nc


def _enable_ldw_opt():
    """walrus emits one LDWEIGHTS per matmul with --enable-ldw-opt=false
    (most are redundant reloads of the same diagonal).  Rewrite the flag on
    the compiler command line."""
    import concourse.bass_utils as BU
    if getattr(BU, "_ldw_patched", False):
        return
    orig = BU.run_command

    def patched(cmd, *a, **kw):
        cmd = [c.replace("--enable-ldw-opt=false", "--enable-ldw-opt=true")
               if isinstance(c, str) else c for c in cmd]
        return orig(cmd, *a, **kw)

    BU.run_command = patched
    BU._ldw_patched = True


_nc_cache = None


def _get_program():
    global _nc_cache
    if _nc_cache is None:
        _enable_ldw_opt()
        _nc_cache = _build_program()
    return _nc_cache


def _host_consts(conv_w, bn_gamma, bn_beta, bn_mean, bn_var):
    s_a = bn_gamma / np.sqrt(bn_var + BN_EPS)
    b72 = (bn_beta - bn_mean * s_a).astype(np.float32).reshape(72, 1)
    A = (conv_w * s_a[:, None]) / np.float32(H * W)
    p = np.arange(128)
    at128 = np.ascontiguousarray(A.T[p // 2]).astype(np.float32)  # (128, 72)
    oc = np.arange(72)
    r9 = (oc[:, None] % 9 == np.arange(9)[None, :]).astype(np.float32)
    g728 = (oc[:, None] // 9 == np.arange(8)[None, :]).astype(np.float32)
    h8128 = (np.arange(8)[:, None] == (p[None, :] // 16)).astype(np.float32)
    eye = np.eye(128, dtype=np.float32)
    return dict(at128=at128, b72=b72, r9=r9, g728=g728, h8128=h8128,
                eye=eye, eyer=eye)


def _shard_pad(x, half):
    """[64, 96(+1 ctx), 192] half-image -> [64, 98, 192] with both vertical
    halos materialized (reflection resolved here)."""
    if half == 0:
        # rows: [reflect(=row 1), 0..95, 96]
        return np.concatenate([x[:, 1:2], x[:, 0:97]], axis=1)
    # rows: [95, 96..191, reflect(=row 190)]
    return np.concatenate([x[:, 95:192], x[:, 190:191]], axis=1)


def _in_maps(x, consts):
    """Core 2j gets (sample 2j, half0) + (sample 2j+1, half1); core 2j+1
    gets (sample 2j, half1) + (sample 2j+1, half0)."""
    maps = []
    for core in range(N):
        j = core // 2
        odd = core % 2
        sa, sb = 2 * j, 2 * j + 1
        xa = _shard_pad(x[sa], odd)
        xb = _shard_pad(x[sb], 1 - odd)
        maps.append(dict(xa=np.ascontiguousarray(xa),
                         xb=np.ascontiguousarray(xb), **consts))
    return maps


def _gather(res):
    low = np.empty((N, IC, H, W), np.float32)
    high = np.empty((N, IC, H, W), np.float32)
    for core in range(N):
        j = core // 2
        odd = core % 2
        sa, sb = 2 * j, 2 * j + 1
        ra = slice(96, 192) if odd else slice(0, 96)
        rb = slice(0, 96) if odd else slice(96, 192)
        low[sa][:, ra] = res[core]["low_a"]
        high[sa][:, ra] = res[core]["high_a"]
        low[sb][:, rb] = res[core]["low_b"]
        high[sb][:, rb] = res[core]["high_b"]
    return low, high


def kernel(x, conv_w, bn_gamma, bn_beta, bn_mean, bn_var):
    x = np.ascontiguousarray(np.asarray(x, dtype=np.float32))
    consts = _host_consts(np.asarray(conv_w, np.float32),
                          np.asarray(bn_gamma, np.float32),
                          np.asarray(bn_beta, np.float32),
                          np.asarray(bn_mean, np.float32),
                          np.asarray(bn_var, np.float32))
    nc = _get_program()
    res = run_bass_kernel_spmd(nc, _in_maps(x, consts),
                               list(range(N))).results
    return _gather(res)


if __name__ == "__main__":
    rng = np.random.default_rng(0)
    demo = dict(
        x=rng.standard_normal((N, IC, H, W), dtype=np.float32),
        conv_w=rng.standard_normal((72, 64)).astype(np.float32),
        bn_gamma=np.ones(72, np.float32),
        bn_beta=np.zeros(72, np.float32),
        bn_mean=rng.standard_normal(72).astype(np.float32) * 0.1,
        bn_var=rng.uniform(0.5, 1.5, 72).astype(np.float32),
    )
    low, high = kernel(**demo)
    print("ok", low.shape, high.shape)


# revision 24
# speedup vs baseline: 1.4340x; 1.4340x over previous
"""Trainium2 Bass kernel for dynamic low-pass filter decomposition.

Module: global-avg-pool -> 1x1 conv -> BN -> softmax over 3x3 taps gives a
per-(sample, group) 3x3 kernel; applied as a reflect-padded depthwise conv
over x; returns (low, x - low).

Sharding: data-parallel over batch n=8 across 8 NeuronCores (1 sample/core).

Per-core layout: partition p = c*2 + h (h = row-half of the image, c =
channel).  Each partition holds 98 rows x 192 cols of its (channel, half)
with one halo row above/below (reflection resolved at DMA time by source row
choice) plus a 1-element front/back pad so tap-shifted views stay in bounds.

Engine split: the TensorEngine accumulates 8 of the 9 taps as diagonal fp32r
matmuls per 512-col PSUM chunk; ScalarE copies the partial out of PSUM;
VectorE adds the 9th (center) tap in-place in SBUF, fixes the reflect
columns at w=0/191 (6 merged ops) and computes high = x - low.  Consts load
on the GpSimd DMA queue so they never queue behind input super-tiles;
warm-up matmuls keep the PE p-state ramped through the input phase; the
1x1-conv/BN/softmax weight chain runs from exact per-chunk partial sums
(fp32) with BN folded into the conv weights on the host.
"""
import sys
import os

sys.path.insert(0, "/opt/trn_rl_repo")

import numpy as np
from contextlib import ExitStack

import concourse.bass as bass
import concourse.tile as tile
from concourse import bacc, mybir
from concourse.bass_utils import run_bass_kernel_spmd

dt = mybir.dt
f32 = dt.float32

KS = 3
GROUP = 8
IC = 64
BN_EPS = 1e-5
N = 8
H = W = 192
RH = 96                 # rows per half-image
NB = 98 * W             # buffer elems per partition (98 rows of 192)
PAD = 1                 # front pad elems (also 1 at the back)
CH = 512                # matmul chunk (one PSUM bank)
# input load chunks (offset, size) in image-region elems, spread over the
# gp, sync and scalar DMA queues with exactly balanced bytes per queue (the
# gp pair is issued ahead of the consts); sizes shrink toward the end so
# the last partial-sum lands early
GP_X = [(0, 3072), (3072, 3072)]
SYNC_X = [(6144, 3072), (12288, 2304), (16896, 768)]
SCAL_X = [(9216, 3072), (14592, 2304), (17664, 768)]
# landing-ordered views for the partial-sum ops (V: gp0+sync, S: gp1+scalar)
V_PARTIALS = [GP_X[0]] + SYNC_X
S_PARTIALS = [GP_X[1]] + SCAL_X
WARM_PER_CHUNK = (2, 2, 2, 2, 2, 1, 1, 1)
WARM_BRIDGE = 4
WARM_CHAIN = 2                 # warm matmuls slotted between chain matmuls
ST_ROWS = [16, 16, 16, 16, 16, 8, 4, 4]   # compute super-tile heights
PE_ALL9 = (6, 7)               # drain STs: PE does all 9 taps (short V tail)
PE_TAPS = (0, 1, 2, 3, 5, 6, 7, 8)
V_TAP = 4                      # center tap, added in-place in SBUF


def _build_program():
    """Trace the SPMD Bass program (same for every core)."""
    nc = bacc.Bacc("TRN2", target_bir_lowering=False, debug=False,
                   num_devices=N)

    x_d = nc.dram_tensor("x", [64, H, W], dt.float32r, kind="ExternalInput")
    at_d = nc.dram_tensor("at128", [128, 72], f32, kind="ExternalInput")
    b_d = nc.dram_tensor("b72", [72, 1], f32, kind="ExternalInput")
    r9_d = nc.dram_tensor("r9", [72, 9], f32, kind="ExternalInput")
    g_d = nc.dram_tensor("g728", [72, 8], f32, kind="ExternalInput")
    h_d = nc.dram_tensor("h8128", [8, 128], f32, kind="ExternalInput")
    eye_d = nc.dram_tensor("eye", [128, 128], f32, kind="ExternalInput")
    eyer_d = nc.dram_tensor("eyer", [128, 128], dt.float32r,
                            kind="ExternalInput")
    low_d = nc.dram_tensor("low", [64, H, W], f32, kind="ExternalOutput")
    high_d = nc.dram_tensor("high", [64, H, W], f32, kind="ExternalOutput")

    xt_dram = x_d.ap()

    def dram_flat(tensor, base, inner):
        """Flat (128, inner) AP over DRAM: partition p = c*2 + h covers
        x.flat[p*18432 + base : ... + inner].  Flat leading-dim-128 APs get
        the full 16-engine DMA spray (~430 GB/s); (h,c)-interleaved ones
        only engage 2 engines (~53 GB/s measured)."""
        return bass.AP(tensor, base, [[RH * W, 128], [1, inner]])

    with tile.TileContext(nc) as tc, ExitStack() as ctx:
        cpool = ctx.enter_context(tc.tile_pool(name="consts", bufs=1))
        xpool = ctx.enter_context(tc.tile_pool(name="x", bufs=1))
        wpool = ctx.enter_context(tc.tile_pool(name="w", bufs=1))
        spool = ctx.enter_context(tc.tile_pool(name="stage", bufs=3))

        # ---- x chunk loads FIRST (queue FIFO position = landing time) on
        # sync+scalar queues; halos behind them on sync; consts go on the
        # GpSimd queue so they land early without delaying x ----
        xt = xpool.tile([128, PAD + NB + 1], dt.float32r)
        # the 1-elem front/back pads are read (never used) by tap-shifted
        # edge views; zero them so they hold no junk/NaN
        nc.vector.memset(xt[:, 0:PAD].bitcast(f32), 0.0)
        nc.vector.memset(xt[:, PAD + NB:PAD + NB + 1].bitcast(f32), 0.0)
        for eng, chunks in ((nc.gpsimd, GP_X), (nc.sync, SYNC_X),
                            (nc.scalar, SCAL_X)):
            for off, sz in chunks:
                eng.dma_start(xt[:, PAD + W + off:PAD + W + off + sz],
                              dram_flat(xt_dram.tensor, off, sz))
        # halo row 97 <- image rows {96, 190 (reflect)}[h]; only needed by
        # the last super-tile, so it queues behind sync's chunks
        nc.sync.dma_start(xt[:, PAD + 97 * W:PAD + 98 * W],
                          bass.AP(xt_dram.tensor, 96 * W,
                                  [[H * W, 64], [94 * W, 2], [1, W]]))

        at_s = cpool.tile([128, 72], f32)
        b_s = cpool.tile([72, 1], f32)
        r9_s = cpool.tile([72, 9], f32)
        g_s = cpool.tile([72, 8], f32)
        h_s = cpool.tile([8, 128], f32)
        eye_s = cpool.tile([128, 128], f32)
        eyer_s = cpool.tile([128, 128], dt.float32r)
        for t, d in ((eyer_s, eyer_d), (at_s, at_d), (b_s, b_d),
                     (r9_s, r9_d), (g_s, g_d), (h_s, h_d), (eye_s, eye_d)):
            nc.gpsimd.dma_start(t[:], d.ap())
        # halo row 0 <- image rows {1 (reflect), 95}[h] (8th gp issue: the gp
        # queue has 8 descriptor slots)
        nc.gpsimd.dma_start(xt[:, PAD:PAD + W],
                            bass.AP(xt_dram.tensor, W,
                                    [[H * W, 64], [94 * W, 2], [1, W]]))

        # ---- partial sums per chunk (VectorE reduce / ScalarE accumulate)
        # into one [128, 7] tile; engines split so both overlap the DMAs ----
        pt = wpool.tile([128, 8], f32)
        rscratch = wpool.tile([128, 3072], f32)
        edummy = wpool.tile([72, 1], f32)
        for vcol, (off, sz) in enumerate(V_PARTIALS):
            src = xt[:, PAD + W + off:PAD + W + off + sz].bitcast(f32)
            nc.vector.tensor_reduce(pt[:, vcol:vcol + 1], src,
                                    axis=mybir.AxisListType.X,
                                    op=mybir.AluOpType.add)
        for scol, (off, sz) in enumerate(S_PARTIALS):
            src = xt[:, PAD + W + off:PAD + W + off + sz].bitcast(f32)
            nc.scalar.activation(rscratch[:, 0:sz], src,
                                 mybir.ActivationFunctionType.Copy,
                                 accum_out=pt[:, 4 + scol:5 + scol])
            if scol == 0:
                # dummy Exp so the activation table holding Exp is loaded
                # during the input phase, not on the chain's critical path
                nc.scalar.activation(edummy[:], b_s[:],
                                     mybir.ActivationFunctionType.Exp)

        # ---- PE warm-up: dummy matmuls chained to each chunk's landing keep
        # the p-state ramped through the otherwise PE-idle input phase ----
        with tc.tile_pool(name="warmp", bufs=1,
                          space=bass.MemorySpace.PSUM) as warmp:
            wrm = warmp.tile([128, CH], f32, tag="wrm")
            all_chunks = [GP_X[0], GP_X[1], SYNC_X[0], SCAL_X[0],
                          SYNC_X[1], SCAL_X[1], SYNC_X[2], SCAL_X[2]]
            for i, (off, sz) in enumerate(all_chunks):
                a = PAD + W + off
                for j in range(WARM_PER_CHUNK[i]):
                    nc.tensor.matmul(wrm[:], eyer_s[:], xt[:, a:a + CH])
            a = PAD + W + all_chunks[-1][0]
            for j in range(WARM_BRIDGE):
                nc.tensor.matmul(wrm[:], eyer_s[:], xt[:, a:a + CH])

        # ---- weight generation chain ----
        sum128 = wpool.tile([128, 1], f32)
        nc.vector.tensor_reduce(sum128[:], pt[:, 0:8],
                                axis=mybir.AxisListType.X,
                                op=mybir.AluOpType.add)
        w128 = wpool.tile([128, 9], f32)
        wc = wpool.tile([128, 3], f32)
        diag = [wpool.tile([128, 128], dt.float32r, name=f"diag{k}")
                for k in range(9)]
        wa = PAD + W

        def warm_chain():
            for j in range(WARM_CHAIN):
                nc.tensor.matmul(wrm2[:], eyer_s[:], xt[:, wa:wa + CH])

        with tc.tile_pool(name="wpsum", bufs=1,
                          space=bass.MemorySpace.PSUM) as wpsum:
            wrm2 = wpsum.tile([128, CH], f32, tag="wrm2")
            lf_p = wpsum.tile([72, 1], f32, tag="lf")
            nc.tensor.matmul(lf_p[:], at_s[:], sum128[:])
            warm_chain()
            e72 = wpool.tile([72, 1], f32)
            nc.scalar.activation(e72[:], lf_p[:],
                                 mybir.ActivationFunctionType.Exp,
                                 bias=b_s[:, 0:1], scale=1.0)
            rhsw = wpool.tile([72, 9], f32)
            nc.vector.tensor_scalar_mul(rhsw[:], r9_s[:], e72[:, 0:1])
            w89_p = wpsum.tile([8, 9], f32, tag="w89")
            nc.tensor.matmul(w89_p[:], g_s[:], rhsw[:])
            warm_chain()
            s8 = wpool.tile([8, 1], f32)
            nc.vector.tensor_reduce(s8[:], w89_p[:],
                                    axis=mybir.AxisListType.X,
                                    op=mybir.AluOpType.add)
            r8 = wpool.tile([8, 1], f32)
            nc.vector.reciprocal(r8[:], s8[:])
            w89s = wpool.tile([8, 9], f32)
            nc.vector.tensor_scalar_mul(w89s[:], w89_p[:], r8[:, 0:1])
            wbig_p = wpsum.tile([128, 9], f32, tag="wbig")
            nc.tensor.matmul(wbig_p[:], h_s[:], w89s[:])
            warm_chain()
            # w128 to SBUF; everything downstream (diags, V tap, edge fixes)
            # reads SBUF so this pool closes without gating the main loop
            nc.scalar.copy(w128[:], wbig_p[:])
        # diagonal weight matrices, in ST0's tap order so the PE never waits
        # on a later diag
        for k in PE_TAPS:
            nc.vector.tensor_scalar_mul(diag[k][:], eye_s[:],
                                        w128[:, k:k + 1])
        nc.vector.tensor_scalar_mul(diag[V_TAP][:], eye_s[:],
                                    w128[:, V_TAP:V_TAP + 1])
        # wc merges the dj=0/dj=2 weights hitting the mirror neighbour of a
        # reflected edge column
        nc.vector.tensor_tensor(wc[:], w128[:, 0:9:3], w128[:, 2:9:3],
                                op=mybir.AluOpType.add)

        # ---- main loop ----
        with tc.tile_pool(name="psum", bufs=8,
                          space=bass.MemorySpace.PSUM) as psum:
            r0 = 0
            for s, rows in enumerate(ST_ROWS):
                stw = rows * W
                base = PAD + W + r0 * W
                chunks = []
                o = 0
                while o < stw:
                    chunks.append((o, min(CH, stw - o)))
                    o += CH

                def tap_view(k, lo, sz):
                    di, dj = k // 3, k % 3
                    off = base + lo + (di - 1) * W + (dj - 1)
                    return xt[:, off:off + sz]

                # tiles are allocated at uniform 512 width (matmuls address
                # a prefix) so the pool's tag slot size never changes
                acc = [psum.tile([128, CH], f32, tag="acc",
                                 name=f"acc{s}_{i}")[:, 0:csz]
                       for i, (co, csz) in enumerate(chunks)]
                all9 = s in PE_ALL9
                pe_taps = tuple(range(9)) if all9 else PE_TAPS
                taps = pe_taps if s % 2 == 0 else pe_taps[::-1]
                for k in taps:
                    for i, (co, csz) in enumerate(chunks):
                        nc.tensor.matmul(acc[i][:], diag[k][:],
                                         tap_view(k, co, csz),
                                         start=(k == taps[0]),
                                         stop=(k == taps[-1]))
                low_st = spool.tile([128, stw], f32, tag="low",
                                    padded_shape=[128, 3072])
                # ScalarE drains PSUM; VectorE adds the center tap in-place
                # in SBUF right behind it, chunk by chunk (drain STs do all
                # 9 taps on the PE so the final V tail stays short)
                for i, (co, csz) in enumerate(chunks):
                    dst = low_st[:, co:co + csz]
                    nc.scalar.copy(dst, acc[i][:])
                    if not all9:
                        nc.vector.scalar_tensor_tensor(
                            dst, tap_view(V_TAP, co, csz).bitcast(f32),
                            w128[:, V_TAP:V_TAP + 1], dst,
                            op0=mybir.AluOpType.mult,
                            op1=mybir.AluOpType.add)
                # edge-column fixes (reflect at w=0 and w=191): per di, the
                # edge output is w[di,1]*x[.,edge] + (w[di,0]+w[di,2])*x[.,
                # mirror-neighbour]; both columns per op via strided views
                out_ap = low_st[:, 0:stw].rearrange(
                    "p (r w) -> p r w", w=W)[:, :, 0:W:W - 1]
                for di in range(3):
                    vb = PAD + (r0 + di) * W
                    va = xt[:, vb:vb + stw].bitcast(f32).rearrange(
                        "p (r w) -> p r w", w=W)[:, :, 0:W:W - 1]
                    vn = xt[:, vb + 1:vb + 1 + stw].bitcast(f32).rearrange(
                        "p (r w) -> p r w", w=W)[:, :, 0:190:189]
                    if di == 0:
                        nc.vector.tensor_scalar_mul(out_ap, va, w128[:, 1:2])
                    else:
                        nc.vector.scalar_tensor_tensor(
                            out_ap, va, w128[:, 3 * di + 1:3 * di + 2],
                            out_ap, op0=mybir.AluOpType.mult,
                            op1=mybir.AluOpType.add)
                    nc.vector.scalar_tensor_tensor(
                        out_ap, vn, wc[:, di:di + 1], out_ap,
                        op0=mybir.AluOpType.mult,
                        op1=mybir.AluOpType.add)
                high_st = spool.tile([128, stw], f32, tag="high",
                                     padded_shape=[128, 3072])
                nc.vector.tensor_tensor(high_st[:],
                                        xt[:, base:base + stw].bitcast(f32),
                                        low_st[:],
                                        op=mybir.AluOpType.subtract)
                nc.scalar.dma_start(
                    dram_flat(low_d.ap().tensor, r0 * W, stw), low_st[:])
                nc.sync.dma_start(
                    dram_flat(high_d.ap().tensor, r0 * W, stw), high_st[:])
                r0 += rows

    nc.compile()
    return nc


def _enable_ldw_opt():
    """walrus emits one LDWEIGHTS per matmul with --enable-ldw-opt=false
    (most are redundant reloads of the same diagonal).  Rewrite the flag on
    the compiler command line."""
    import concourse.bass_utils as BU
    if getattr(BU, "_ldw_patched", False):
        return
    orig = BU.run_command

    def patched(cmd, *a, **kw):
        cmd = [c.replace("--enable-ldw-opt=false", "--enable-ldw-opt=true")
               if isinstance(c, str) else c for c in cmd]
        return orig(cmd, *a, **kw)

    BU.run_command = patched
    BU._ldw_patched = True


_nc_cache = None


def _get_program():
    global _nc_cache
    if _nc_cache is None:
        _enable_ldw_opt()
        _nc_cache = _build_program()
    return _nc_cache


def _host_consts(conv_w, bn_gamma, bn_beta, bn_mean, bn_var):
    s_a = bn_gamma / np.sqrt(bn_var + BN_EPS)
    b72 = (bn_beta - bn_mean * s_a).astype(np.float32).reshape(72, 1)
    A = (conv_w * s_a[:, None]) / np.float32(H * W)
    p = np.arange(128)
    at128 = np.ascontiguousarray(A.T[p // 2]).astype(np.float32)  # (128, 72)
    oc = np.arange(72)
    r9 = (oc[:, None] % 9 == np.arange(9)[None, :]).astype(np.float32)
    g728 = (oc[:, None] // 9 == np.arange(8)[None, :]).astype(np.float32)
    h8128 = (np.arange(8)[:, None] == (p[None, :] // 16)).astype(np.float32)
    eye = np.eye(128, dtype=np.float32)
    return dict(at128=at128, b72=b72, r9=r9, g728=g728, h8128=h8128,
                eye=eye, eyer=eye)


def kernel(x, conv_w, bn_gamma, bn_beta, bn_mean, bn_var):
    x = np.ascontiguousarray(np.asarray(x, dtype=np.float32))
    consts = _host_consts(np.asarray(conv_w, np.float32),
                          np.asarray(bn_gamma, np.float32),
                          np.asarray(bn_beta, np.float32),
                          np.asarray(bn_mean, np.float32),
                          np.asarray(bn_var, np.float32))
    nc = _get_program()
    in_maps = [dict(x=x[i], **consts) for i in range(N)]
    res = run_bass_kernel_spmd(nc, in_maps, list(range(N))).results
    low = np.stack([res[i]["low"] for i in range(N)])
    high = np.stack([res[i]["high"] for i in range(N)])
    return low, high


if __name__ == "__main__":
    rng = np.random.default_rng(0)
    demo = dict(
        x=rng.standard_normal((N, IC, H, W), dtype=np.float32),
        conv_w=rng.standard_normal((72, 64)).astype(np.float32),
        bn_gamma=np.ones(72, np.float32),
        bn_beta=np.zeros(72, np.float32),
        bn_mean=rng.standard_normal(72).astype(np.float32) * 0.1,
        bn_var=rng.uniform(0.5, 1.5, 72).astype(np.float32),
    )
    low, high = kernel(**demo)
    print("ok", low.shape, high.shape)


# revision 25
# speedup vs baseline: 1.4861x; 1.0363x over previous
"""Trainium2 Bass kernel for dynamic low-pass filter decomposition.

Module: global-avg-pool -> 1x1 conv -> BN -> softmax over 3x3 taps gives a
per-(sample, group) 3x3 kernel; applied as a reflect-padded depthwise conv
over x; returns (low, x - low).

Sharding: data-parallel over batch n=8 across 8 NeuronCores (1 sample/core).

Per-core layout: partition p = c*2 + h (h = row-half of the image, c =
channel).  Each partition holds 98 rows x 192 cols of its (channel, half)
with one halo row above/below (reflection resolved at DMA time by source row
choice) plus a 1-element front/back pad so tap-shifted views stay in bounds.

Engine split: the TensorEngine accumulates 8 of the 9 taps as diagonal fp32r
matmuls per 512-col PSUM chunk; ScalarE copies the partial out of PSUM;
VectorE adds the 9th (center) tap in-place in SBUF, fixes the reflect
columns at w=0/191 (6 merged ops) and computes high = x - low.  Consts load
on the GpSimd DMA queue so they never queue behind input super-tiles;
warm-up matmuls keep the PE p-state ramped through the input phase; the
1x1-conv/BN/softmax weight chain runs from exact per-chunk partial sums
(fp32) with BN folded into the conv weights on the host.
"""
import sys
import os

sys.path.insert(0, "/opt/trn_rl_repo")

import numpy as np
from contextlib import ExitStack

import concourse.bass as bass
import concourse.tile as tile
from concourse import bacc, mybir
from concourse.bass_utils import run_bass_kernel_spmd

dt = mybir.dt
f32 = dt.float32

KS = 3
GROUP = 8
IC = 64
BN_EPS = 1e-5
N = 8
H = W = 192
RH = 96                 # rows per half-image
NB = 98 * W             # buffer elems per partition (98 rows of 192)
PAD = 1                 # front pad elems (also 1 at the back)
CH = 512                # matmul chunk (one PSUM bank)
# input load chunks (offset, size) in image-region elems, alternating
# between the sync and scalar DMA queues with exactly balanced bytes per
# queue; sizes shrink toward the end so the last partial-sum lands early
LOAD_CHUNKS = [(0, 3072), (3072, 3072), (6144, 3072), (9216, 3072),
               (12288, 2304), (14592, 2304), (16896, 768), (17664, 768)]
SYNC_LOADS = (0, 2, 4, 6)      # chunk idx -> sync queue, rest on scalar
WARM_PER_CHUNK = (2, 2, 2, 2, 2, 1, 1, 1)
WARM_BRIDGE = 4
WARM_CHAIN = 2                 # warm matmuls slotted between chain matmuls
ST_ROWS = [16, 16, 16, 16, 16, 8, 4, 4]   # compute super-tile heights
PE_ALL9 = (6, 7)               # drain STs: PE does all 9 taps (short V tail)
PE_TAPS = (0, 1, 2, 3, 5, 6, 7, 8)
V_TAP = 4                      # center tap, added in-place in SBUF


def _build_program():
    """Trace the SPMD Bass program (same for every core)."""
    nc = bacc.Bacc("TRN2", target_bir_lowering=False, debug=False,
                   num_devices=N)

    x_d = nc.dram_tensor("x", [64, H, W], dt.float32r, kind="ExternalInput")
    at_d = nc.dram_tensor("at128", [128, 72], f32, kind="ExternalInput")
    b_d = nc.dram_tensor("b72", [72, 1], f32, kind="ExternalInput")
    r9_d = nc.dram_tensor("r9", [72, 9], f32, kind="ExternalInput")
    g_d = nc.dram_tensor("g728", [72, 8], f32, kind="ExternalInput")
    h_d = nc.dram_tensor("h8128", [8, 128], f32, kind="ExternalInput")
    eye_d = nc.dram_tensor("eye", [128, 128], f32, kind="ExternalInput")
    eyer_d = nc.dram_tensor("eyer", [128, 128], dt.float32r,
                            kind="ExternalInput")
    low_d = nc.dram_tensor("low", [64, H, W], f32, kind="ExternalOutput")
    high_d = nc.dram_tensor("high", [64, H, W], f32, kind="ExternalOutput")

    xt_dram = x_d.ap()

    def dram_flat(tensor, base, inner):
        """Flat (128, inner) AP over DRAM: partition p = c*2 + h covers
        x.flat[p*18432 + base : ... + inner].  Flat leading-dim-128 APs get
        the full 16-engine DMA spray (~430 GB/s); (h,c)-interleaved ones
        only engage 2 engines (~53 GB/s measured)."""
        return bass.AP(tensor, base, [[RH * W, 128], [1, inner]])

    with tile.TileContext(nc) as tc, ExitStack() as ctx:
        cpool = ctx.enter_context(tc.tile_pool(name="consts", bufs=1))
        xpool = ctx.enter_context(tc.tile_pool(name="x", bufs=1))
        wpool = ctx.enter_context(tc.tile_pool(name="w", bufs=1))
        spool = ctx.enter_context(tc.tile_pool(name="stage", bufs=3))

        # ---- x chunk loads FIRST (queue FIFO position = landing time) on
        # sync+scalar queues; halos behind them on sync; consts go on the
        # GpSimd queue so they land early without delaying x ----
        xt = xpool.tile([128, PAD + NB + 1], dt.float32r)
        # the 1-elem front/back pads are read (never used) by tap-shifted
        # edge views; zero them so they hold no junk/NaN
        nc.vector.memset(xt[:, 0:PAD].bitcast(f32), 0.0)
        nc.vector.memset(xt[:, PAD + NB:PAD + NB + 1].bitcast(f32), 0.0)
        for i, (off, sz) in enumerate(LOAD_CHUNKS):
            eng = nc.sync if i in SYNC_LOADS else nc.scalar
            eng.dma_start(xt[:, PAD + W + off:PAD + W + off + sz],
                          dram_flat(xt_dram.tensor, off, sz))
        # halo row 97 <- image rows {96, 190 (reflect)}[h]; only needed by
        # the last super-tile, so it queues behind sync's chunks
        nc.sync.dma_start(xt[:, PAD + 97 * W:PAD + 98 * W],
                          bass.AP(xt_dram.tensor, 96 * W,
                                  [[H * W, 64], [94 * W, 2], [1, W]]))

        at_s = cpool.tile([128, 72], f32)
        b_s = cpool.tile([72, 1], f32)
        r9_s = cpool.tile([72, 9], f32)
        g_s = cpool.tile([72, 8], f32)
        h_s = cpool.tile([8, 128], f32)
        eye_s = cpool.tile([128, 128], f32)
        eyer_s = cpool.tile([128, 128], dt.float32r)
        for t, d in ((eyer_s, eyer_d), (at_s, at_d), (b_s, b_d),
                     (r9_s, r9_d), (g_s, g_d), (h_s, h_d), (eye_s, eye_d)):
            nc.gpsimd.dma_start(t[:], d.ap())
        # halo row 0 <- image rows {1 (reflect), 95}[h] (8th gp issue: the gp
        # queue has 8 descriptor slots)
        nc.gpsimd.dma_start(xt[:, PAD:PAD + W],
                            bass.AP(xt_dram.tensor, W,
                                    [[H * W, 64], [94 * W, 2], [1, W]]))

        # ---- partial sums per chunk (VectorE reduce / ScalarE accumulate)
        # into one [128, 7] tile; engines split so both overlap the DMAs ----
        pt = wpool.tile([128, 8], f32)
        rscratch = wpool.tile([128, 3072], f32)
        edummy = wpool.tile([72, 1], f32)
        vcol = scol = 0
        for i, (off, sz) in enumerate(LOAD_CHUNKS):
            src = xt[:, PAD + W + off:PAD + W + off + sz].bitcast(f32)
            if i in SYNC_LOADS:
                nc.vector.tensor_reduce(pt[:, vcol:vcol + 1], src,
                                        axis=mybir.AxisListType.X,
                                        op=mybir.AluOpType.add)
                vcol += 1
            else:
                nc.scalar.activation(rscratch[:, 0:sz], src,
                                     mybir.ActivationFunctionType.Copy,
                                     accum_out=pt[:, 4 + scol:5 + scol])
                scol += 1
                if scol == 1:
                    # dummy Exp so the activation table holding Exp is
                    # loaded during the input phase, not on the chain's
                    # critical path
                    nc.scalar.activation(edummy[:], b_s[:],
                                         mybir.ActivationFunctionType.Exp)

        # ---- PE warm-up: dummy matmuls chained to each chunk's landing keep
        # the p-state ramped through the otherwise PE-idle input phase ----
        with tc.tile_pool(name="warmp", bufs=1,
                          space=bass.MemorySpace.PSUM) as warmp:
            wrm = warmp.tile([128, CH], f32, tag="wrm")
            for i, (off, sz) in enumerate(LOAD_CHUNKS):
                a = PAD + W + off
                for j in range(WARM_PER_CHUNK[i]):
                    nc.tensor.matmul(wrm[:], eyer_s[:], xt[:, a:a + CH])
            a = PAD + W + LOAD_CHUNKS[-1][0]
            for j in range(WARM_BRIDGE):
                nc.tensor.matmul(wrm[:], eyer_s[:], xt[:, a:a + CH])

        # ---- weight generation chain ----
        sum128 = wpool.tile([128, 1], f32)
        nc.vector.tensor_reduce(sum128[:], pt[:, 0:8],
                                axis=mybir.AxisListType.X,
                                op=mybir.AluOpType.add)
        w128 = wpool.tile([128, 9], f32)
        wc = wpool.tile([128, 3], f32)
        diag = [wpool.tile([128, 128], dt.float32r, name=f"diag{k}")
                for k in range(9)]
        wa = PAD + W

        def warm_chain():
            for j in range(WARM_CHAIN):
                nc.tensor.matmul(wrm2[:], eyer_s[:], xt[:, wa:wa + CH])

        with tc.tile_pool(name="wpsum", bufs=1,
                          space=bass.MemorySpace.PSUM) as wpsum:
            wrm2 = wpsum.tile([128, CH], f32, tag="wrm2")
            lf_p = wpsum.tile([72, 1], f32, tag="lf")
            nc.tensor.matmul(lf_p[:], at_s[:], sum128[:])
            warm_chain()
            e72 = wpool.tile([72, 1], f32)
            nc.scalar.activation(e72[:], lf_p[:],
                                 mybir.ActivationFunctionType.Exp,
                                 bias=b_s[:, 0:1], scale=1.0)
            rhsw = wpool.tile([72, 9], f32)
            nc.vector.tensor_scalar_mul(rhsw[:], r9_s[:], e72[:, 0:1])
            w89_p = wpsum.tile([8, 9], f32, tag="w89")
            nc.tensor.matmul(w89_p[:], g_s[:], rhsw[:])
            warm_chain()
            s8 = wpool.tile([8, 1], f32)
            nc.vector.tensor_reduce(s8[:], w89_p[:],
                                    axis=mybir.AxisListType.X,
                                    op=mybir.AluOpType.add)
            r8 = wpool.tile([8, 1], f32)
            nc.vector.reciprocal(r8[:], s8[:])
            w89s = wpool.tile([8, 9], f32)
            nc.vector.tensor_scalar_mul(w89s[:], w89_p[:], r8[:, 0:1])
            wbig_p = wpsum.tile([128, 9], f32, tag="wbig")
            nc.tensor.matmul(wbig_p[:], h_s[:], w89s[:])
            warm_chain()
            # w128 to SBUF; everything downstream (diags, V tap, edge fixes)
            # reads SBUF so this pool closes without gating the main loop
            nc.scalar.copy(w128[:], wbig_p[:])
        # diagonal weight matrices, in ST0's tap order so the PE never waits
        # on a later diag
        for k in PE_TAPS:
            nc.vector.tensor_scalar_mul(diag[k][:], eye_s[:],
                                        w128[:, k:k + 1])
        nc.vector.tensor_scalar_mul(diag[V_TAP][:], eye_s[:],
                                    w128[:, V_TAP:V_TAP + 1])
        # wc merges the dj=0/dj=2 weights hitting the mirror neighbour of a
        # reflected edge column
        nc.vector.tensor_tensor(wc[:], w128[:, 0:9:3], w128[:, 2:9:3],
                                op=mybir.AluOpType.add)

        # ---- main loop ----
        with tc.tile_pool(name="psum", bufs=8,
                          space=bass.MemorySpace.PSUM) as psum:
            r0 = 0
            for s, rows in enumerate(ST_ROWS):
                stw = rows * W
                base = PAD + W + r0 * W
                chunks = []
                o = 0
                while o < stw:
                    chunks.append((o, min(CH, stw - o)))
                    o += CH

                def tap_view(k, lo, sz):
                    di, dj = k // 3, k % 3
                    off = base + lo + (di - 1) * W + (dj - 1)
                    return xt[:, off:off + sz]

                acc = [psum.tile([128, csz], f32, tag="acc",
                                 name=f"acc{s}_{i}")
                       for i, (co, csz) in enumerate(chunks)]
                all9 = s in PE_ALL9
                pe_taps = tuple(range(9)) if all9 else PE_TAPS
                taps = pe_taps if s % 2 == 0 else pe_taps[::-1]
                for k in taps:
                    for i, (co, csz) in enumerate(chunks):
                        nc.tensor.matmul(acc[i][:], diag[k][:],
                                         tap_view(k, co, csz),
                                         start=(k == taps[0]),
                                         stop=(k == taps[-1]))
                low_st = spool.tile([128, stw], f32, tag="low",
                                    padded_shape=[128, 3072])
                # ScalarE drains PSUM; VectorE adds the center tap in-place
                # in SBUF right behind it, chunk by chunk (drain STs do all
                # 9 taps on the PE so the final V tail stays short)
                for i, (co, csz) in enumerate(chunks):
                    dst = low_st[:, co:co + csz]
                    nc.scalar.copy(dst, acc[i][:])
                    if not all9:
                        nc.vector.scalar_tensor_tensor(
                            dst, tap_view(V_TAP, co, csz).bitcast(f32),
                            w128[:, V_TAP:V_TAP + 1], dst,
                            op0=mybir.AluOpType.mult,
                            op1=mybir.AluOpType.add)
                # edge-column fixes (reflect at w=0 and w=191): per di, the
                # edge output is w[di,1]*x[.,edge] + (w[di,0]+w[di,2])*x[.,
                # mirror-neighbour]; both columns per op via strided views
                out_ap = low_st[:, 0:stw].rearrange(
                    "p (r w) -> p r w", w=W)[:, :, 0:W:W - 1]
                for di in range(3):
                    vb = PAD + (r0 + di) * W
                    va = xt[:, vb:vb + stw].bitcast(f32).rearrange(
                        "p (r w) -> p r w", w=W)[:, :, 0:W:W - 1]
                    vn = xt[:, vb + 1:vb + 1 + stw].bitcast(f32).rearrange(
                        "p (r w) -> p r w", w=W)[:, :, 0:190:189]
                    if di == 0:
                        nc.vector.tensor_scalar_mul(out_ap, va, w128[:, 1:2])
                    else:
                        nc.vector.scalar_tensor_tensor(
                            out_ap, va, w128[:, 3 * di + 1:3 * di + 2],
                            out_ap, op0=mybir.AluOpType.mult,
                            op1=mybir.AluOpType.add)
                    nc.vector.scalar_tensor_tensor(
                        out_ap, vn, wc[:, di:di + 1], out_ap,
                        op0=mybir.AluOpType.mult,
                        op1=mybir.AluOpType.add)
                high_st = spool.tile([128, stw], f32, tag="high",
                                     padded_shape=[128, 3072])
                nc.vector.tensor_tensor(high_st[:],
                                        xt[:, base:base + stw].bitcast(f32),
                                        low_st[:],
                                        op=mybir.AluOpType.subtract)
                nc.scalar.dma_start(
                    dram_flat(low_d.ap().tensor, r0 * W, stw), low_st[:])
                nc.sync.dma_start(
                    dram_flat(high_d.ap().tensor, r0 * W, stw), high_st[:])
                r0 += rows

    nc.compile()
    return nc


def _enable_ldw_opt():
    """walrus emits one LDWEIGHTS per matmul with --enable-ldw-opt=false
    (most are redundant reloads of the same diagonal).  Rewrite the flag on
    the compiler command line."""
    import concourse.bass_utils as BU
    if getattr(BU, "_ldw_patched", False):
        return
    orig = BU.run_command

    def patched(cmd, *a, **kw):
        cmd = [c.replace("--enable-ldw-opt=false", "--enable-ldw-opt=true")
               if isinstance(c, str) else c for c in cmd]
        return orig(cmd, *a, **kw)

    BU.run_command = patched
    BU._ldw_patched = True


_nc_cache = None


def _get_program():
    global _nc_cache
    if _nc_cache is None:
        _enable_ldw_opt()
        _nc_cache = _build_program()
    return _nc_cache


def _host_consts(conv_w, bn_gamma, bn_beta, bn_mean, bn_var):
    s_a = bn_gamma / np.sqrt(bn_var + BN_EPS)
    b72 = (bn_beta - bn_mean * s_a).astype(np.float32).reshape(72, 1)
    A = (conv_w * s_a[:, None]) / np.float32(H * W)
    p = np.arange(128)
    at128 = np.ascontiguousarray(A.T[p // 2]).astype(np.float32)  # (128, 72)
    oc = np.arange(72)
    r9 = (oc[:, None] % 9 == np.arange(9)[None, :]).astype(np.float32)
    g728 = (oc[:, None] // 9 == np.arange(8)[None, :]).astype(np.float32)
    h8128 = (np.arange(8)[:, None] == (p[None, :] // 16)).astype(np.float32)
    eye = np.eye(128, dtype=np.float32)
    return dict(at128=at128, b72=b72, r9=r9, g728=g728, h8128=h8128,
                eye=eye, eyer=eye)


def kernel(x, conv_w, bn_gamma, bn_beta, bn_mean, bn_var):
    x = np.ascontiguousarray(np.asarray(x, dtype=np.float32))
    consts = _host_consts(np.asarray(conv_w, np.float32),
                          np.asarray(bn_gamma, np.float32),
                          np.asarray(bn_beta, np.float32),
                          np.asarray(bn_mean, np.float32),
                          np.asarray(bn_var, np.float32))
    nc = _get_program()
    in_maps = [dict(x=x[i], **consts) for i in range(N)]
    res = run_bass_kernel_spmd(nc, in_maps, list(range(N))).results
    low = np.stack([res[i]["low"] for i in range(N)])
    high = np.stack([res[i]["high"] for i in range(N)])
    return low, high


if __name__ == "__main__":
    rng = np.random.default_rng(0)
    demo = dict(
        x=rng.standard_normal((N, IC, H, W), dtype=np.float32),
        conv_w=rng.standard_normal((72, 64)).astype(np.float32),
        bn_gamma=np.ones(72, np.float32),
        bn_beta=np.zeros(72, np.float32),
        bn_mean=rng.standard_normal(72).astype(np.float32) * 0.1,
        bn_var=rng.uniform(0.5, 1.5, 72).astype(np.float32),
    )
    low, high = kernel(**demo)
    print("ok", low.shape, high.shape)
